# revision 11
# baseline (speedup 1.0000x reference)
"""Trainium2 Bass kernel for nn_DM_35210141892754 (4-direction VMamba block).

Sharding: 8 cores = B(2) x directions(4); each core processes one gathered
sequence (C=64, L=8192) end-to-end. Gather/scatter (reference _rcds/_merge)
run on host with numpy strided ops; all FLOPs run on device.

v2: T=1024 chunks, Pool(gpsimd) offload for muls + partition_broadcast,
Silu activation tables for the gates, bf16 matmuls on the cheap paths.
"""

import sys
import math

sys.path.insert(0, "/opt/trn_rl_repo")

import numpy as np

B, C, HIMG, WIMG = 2, 64, 128, 128
DEPTH = 4
D_STATE, D_CONV, EXPAND = 16, 4, 2
D_INNER = EXPAND * C  # 128
DT_RANK = math.ceil(C / 16)  # 4
L = 8192  # sequence length for step_size=2
T = 1024  # device chunk size
NCH = L // T  # 8
H2 = T // 512  # psum halves per chunk
EPS = 1e-5
NS = D_STATE

OFF = {0: ((1, 0), (0, 0), (0, 1), (1, 1)),
       1: ((0, 0), (1, 0), (1, 1), (0, 1)),
       2: ((0, 1), (1, 1), (1, 0), (0, 0)),
       3: ((1, 1), (0, 1), (0, 0), (1, 0))}

_PROG = {}


# ----------------------------------------------------------------------------
# host-side gather (reference _rcds) and scatter (reference _merge), numpy
# ----------------------------------------------------------------------------

def _rcds_np(f0, f1, s, i):
    fr = np.concatenate([f0, f1], axis=3)
    fl = np.concatenate([f1, f0], axis=3)
    fb = np.concatenate([f0, f1], axis=2)
    ft = np.concatenate([f1, f0], axis=2)
    Bb, Cc = fr.shape[:2]
    r, l, b, t = OFF[i]
    y0 = fr[:, :, r[0]::s, r[1]::s].transpose(0, 1, 3, 2).reshape(Bb, Cc, -1)
    y1 = fl[:, :, l[0]::s, l[1]::s].transpose(0, 1, 3, 2).reshape(Bb, Cc, -1)[:, :, ::-1]
    y2 = fb[:, :, b[0]::s, b[1]::s].reshape(Bb, Cc, -1)
    y3 = ft[:, :, t[0]::s, t[1]::s].reshape(Bb, Cc, -1)[:, :, ::-1]
    feats = np.stack([y0, y1, y2, y3], axis=1)  # (B, 4, C, L)
    return np.ascontiguousarray(feats), fr.shape[2], fb.shape[3]


def _merge_np(ys, ori_h, ori_w, s, i):
    # ys: (B, 4, C, L)
    Bb, K, Cc, Ll = ys.shape
    Hh = -(-ori_h // s)
    Ww = -(-ori_w // s)
    nh, nw = Hh * s, Ww * s
    r, l, b, t = OFF[i]
    y2wr = np.zeros((Bb, Cc, nh, 2 * nw), ys.dtype)
    y2wl = np.zeros((Bb, Cc, nh, 2 * nw), ys.dtype)
    y2hb = np.zeros((Bb, Cc, 2 * nh, nw), ys.dtype)
    y2ht = np.zeros((Bb, Cc, 2 * nh, nw), ys.dtype)
    y2wr[:, :, r[0]::s, r[1]::s] = ys[:, 0].reshape(Bb, Cc, 2 * Ww, Hh).transpose(0, 1, 3, 2)
    y2wl[:, :, l[0]::s, l[1]::s] = ys[:, 1][:, :, ::-1].reshape(Bb, Cc, 2 * Ww, Hh).transpose(0, 1, 3, 2)
    y2hb[:, :, b[0]::s, b[1]::s] = ys[:, 2].reshape(Bb, Cc, 2 * Hh, Ww)
    y2ht[:, :, t[0]::s, t[1]::s] = ys[:, 3][:, :, ::-1].reshape(Bb, Cc, 2 * Hh, Ww)
    if ori_h != nh or ori_w != nw:
        y2wr = y2wr[:, :, :ori_h, :ori_w]
        y2wl = y2wl[:, :, :ori_h, :ori_w]
        y2ht = y2ht[:, :, :ori_h, :ori_w]
        y2hb = y2hb[:, :, :ori_h, :ori_w]
    d0r, d1r = np.split(y2wr, 2, axis=3)
    d1l, d0l = np.split(y2wl, 2, axis=3)
    d0b, d1b = np.split(y2hb, 2, axis=2)
    d1t, d0t = np.split(y2ht, 2, axis=2)
    return d0r + d0l + d0b + d0t, d1r + d1l + d1b + d1t


# ----------------------------------------------------------------------------
# device program
# ----------------------------------------------------------------------------

def _build_program():
    import concourse.bacc as bacc
    import concourse.bass_isa as bass_isa
    import concourse.mybir as mybir
    import concourse.tile as tile

    dt_ = mybir.dt
    F32, BF16, F32R = dt_.float32, dt_.bfloat16, dt_.float32r
    AF = mybir.ActivationFunctionType
    OP = mybir.AluOpType

    def r32(ap):
        return ap.bitcast(F32R)

    nc = bacc.Bacc("TRN2", target_bir_lowering=False, debug=False)

    def din(name, shape, d=F32):
        return nc.dram_tensor(name, shape, d, kind="ExternalInput")

    x_d = din("x", [C, L])
    W2T_d = din("W2T", [C, 2 * D_INNER])
    w1n_d = din("w1n", [1, 2 * D_INNER])
    diag_d = din("diag", [D_INNER, 4 * D_INNER], dt_.bfloat16)
    cbp_d = din("cbp", [D_INNER, 1])
    bzp_d = din("bzp", [D_INNER, 1])
    Wxp_d = din("Wxp", [D_INNER, DT_RANK + 2 * NS], dt_.bfloat16)
    WdtT_d = din("WdtT", [DT_RANK, D_INNER], dt_.bfloat16)
    dtb_d = din("dtb", [D_INNER, 1])
    Aneg_d = din("Aneg", [D_INNER, NS])
    Dsk_d = din("Dsk", [D_INNER, 1])
    WoutT_d = din("WoutT", [D_INNER, C], dt_.bfloat16)
    Wfc1T_d = din("Wfc1T", [C, 4 * C], dt_.bfloat16)
    w1f_d = din("w1f", [1, 4 * C])
    bfc1_d = din("bfc1", [4 * C // 2, 2])
    Wfc2a_d = din("Wfc2a", [2 * C, C], dt_.bfloat16)
    Wfc2b_d = din("Wfc2b", [2 * C, C], dt_.bfloat16)
    mv_d = din("mv", [C, 1])
    ones1_d = din("ones1", [1, C])
    epsb_d = din("epsb", [NCH, 1])
    I128_d = din("I128", [D_INNER, D_INNER], dt_.bfloat16)

    out_d = nc.dram_tensor("out", [C, L], F32, kind="ExternalOutput")

    with tile.TileContext(nc) as tc:
        with tc.tile_pool(name="pers", bufs=1) as pers, \
             tc.tile_pool(name="wka", bufs=2) as wka, \
             tc.tile_pool(name="wkb", bufs=3) as wkb, \
             tc.tile_pool(name="wk2", bufs=1) as wk2, \
             tc.tile_pool(name="hp", bufs=1) as hp, \
             tc.tile_pool(name="cvp", bufs=2) as cvp, \
             tc.tile_pool(name="ps", bufs=3, space="PSUM") as ps, \
             tc.tile_pool(name="pacc", bufs=1, space="PSUM") as pacc, \
             tc.tile_pool(name="dr", bufs=1, space="DRAM") as dr:

            def ld(dram, shape, d=F32, tag=None, rr=False):
                t_ = pers.tile(shape, F32R if rr else d, tag=tag)
                if rr:
                    nc.sync.dma_start(t_[:], r32(dram[:]))
                else:
                    nc.sync.dma_start(t_[:], dram[:])
                return t_

            W2T = ld(W2T_d, [C, 2 * D_INNER], tag="W2T", rr=True)
            w1n = ld(w1n_d, [1, 2 * D_INNER], tag="w1n", rr=True)
            diag = ld(diag_d, [D_INNER, 4 * D_INNER], BF16, tag="diag")
            cbp = ld(cbp_d, [D_INNER, 1], tag="cbp")
            bzp = ld(bzp_d, [D_INNER, 1], tag="bzp")
            Wxp = ld(Wxp_d, [D_INNER, DT_RANK + 2 * NS], BF16, tag="Wxp")
            WdtT = ld(WdtT_d, [DT_RANK, D_INNER], BF16, tag="WdtT")
            dtb = ld(dtb_d, [D_INNER, 1], tag="dtb")
            Aneg = ld(Aneg_d, [D_INNER, NS], tag="Aneg")
            Dsk = ld(Dsk_d, [D_INNER, 1], tag="Dsk")
            WoutT = ld(WoutT_d, [D_INNER, C], BF16, tag="WoutT")
            Wfc1T = ld(Wfc1T_d, [C, 4 * C], BF16, tag="Wfc1T")
            w1f = ld(w1f_d, [1, 4 * C], tag="w1f", rr=True)
            bfc1 = ld(bfc1_d, [4 * C // 2, 2], tag="bfc1")
            Wfc2a = ld(Wfc2a_d, [2 * C, C], BF16, tag="Wfc2a")
            Wfc2b = ld(Wfc2b_d, [2 * C, C], BF16, tag="Wfc2b")
            mv = ld(mv_d, [C, 1], tag="mv", rr=True)
            ones1 = ld(ones1_d, [1, C], tag="ones1", rr=True)
            epsb = ld(epsb_d, [NCH, 1], tag="epsb")
            I128 = ld(I128_d, [D_INNER, D_INNER], BF16, tag="I128")
            invc = pers.tile([NCH, 1], F32, tag="invc")
            nc.vector.memset(invc[:], 1.0 / C)
            invone = pers.tile([NCH, 1], F32, tag="invone")
            nc.vector.memset(invone[:], 1.0)

            fbuf = pers.tile([C, L], F32, tag="fbuf")

            st1_dram = dr.tile([2 * NCH, T], F32, tag="st1d")
            r1_dram = dr.tile([1, L], F32, tag="r1d")
            q1_dram = dr.tile([1, L], F32, tag="q1d")
            st2_dram = dr.tile([2 * NCH, T], F32, tag="st2d")
            bcd_dram = dr.tile([2 * NS, L], dt_.bfloat16, tag="bcd")
            r2_dram = dr.tile([1, L], F32, tag="r2d")
            q2_dram = dr.tile([1, L], F32, tag="q2d")

            HLF = [slice(h * 512, (h + 1) * 512) for h in range(H2)]

            def ln_stats(src_sb, src_f2, st_dram, j):
                # channel sums of a [C,T] tile via gpsimd -> DRAM rows j, NCH+j
                s1 = wka.tile([C, T], F32, tag="s1", bufs=1)
                nc.gpsimd.partition_all_reduce(s1[:], src_sb[:, 0:T], channels=C,
                                               reduce_op=bass_isa.ReduceOp.add)
                s2 = wka.tile([C, T], F32, tag="s2", bufs=1)
                nc.gpsimd.partition_all_reduce(s2[:], src_f2[:], channels=C,
                                               reduce_op=bass_isa.ReduceOp.add)
                nc.sync.dma_start(st_dram[j:j + 1, :], s1[0:1, :])
                nc.sync.dma_start(st_dram[NCH + j:NCH + j + 1, :], s2[0:1, :])

            # ================= phase 0: LN1 stats =================
            for j in range(NCH):
                sl = slice(j * T, (j + 1) * T)
                xj = wka.tile([C, T], F32, tag="xj")
                nc.sync.dma_start(r32(xj[:]), r32(x_d[:, sl]))
                x2 = wka.tile([C, T], F32, tag="xr")
                nc.scalar.activation(r32(x2[:]), xj[:], AF.Square)
                ln_stats(xj, x2, st1_dram, j)

            def stats_finish(st_dram, rf_dram, qf_dram, sums=True):
                sq = (1.0 / C) if sums else 1.0
                col = invc if sums else invone
                mu_all = wk2.tile([NCH, T], F32, tag="sa")
                nc.sync.dma_start(mu_all[:], st_dram[0:NCH, :])
                ms_all = wk2.tile([NCH, T], F32, tag="sb")
                nc.sync.dma_start(ms_all[:], st_dram[NCH:2 * NCH, :])
                t1 = wk2.tile([NCH, T], F32, tag="sc")
                nc.scalar.activation(t1[:], mu_all[:], AF.Square, scale=sq)
                t2 = wk2.tile([NCH, T], F32, tag="sd")
                nc.vector.scalar_tensor_tensor(t2[:], ms_all[:], col[:], t1[:],
                                               OP.mult, OP.subtract)
                t3 = wk2.tile([NCH, T], F32, tag="sc")
                nc.scalar.activation(t3[:], t2[:], AF.Ln, bias=epsb[:])
                rstd = wk2.tile([NCH, T], F32, tag="sd")
                nc.scalar.activation(rstd[:], t3[:], AF.Exp, bias=0.0, scale=-0.5)
                mq = wk2.tile([NCH, T], F32, tag="sc")
                nc.vector.scalar_tensor_tensor(mq[:], mu_all[:], col[:], rstd[:],
                                               OP.mult, OP.mult)
                nc.sync.dma_start(rf_dram[:].rearrange("a (c t) -> (a c) t", t=T), rstd[:])
                nc.sync.dma_start(qf_dram[:].rearrange("a (c t) -> (a c) t", t=T), mq[:])

            stats_finish(st1_dram, r1_dram, q1_dram)

            # ================= steady phase =================
            hbuf = hp.tile([D_INNER, NS, T], BF16, tag="h")
            prev_cv = None
            for j in range(NCH):
                sl = slice(j * T, (j + 1) * T)
                xj = wka.tile([C, T], F32, tag="xj")
                nc.sync.dma_start(xj[:], x_d[:, sl])
                rft = wka.tile([1, T], F32, tag="rft")
                nc.sync.dma_start(r32(rft[:]), r32(r1_dram[0:1, sl]))
                qft = wka.tile([1, T], F32, tag="qft")
                nc.sync.dma_start(r32(qft[:]), r32(q1_dram[0:1, sl]))

                rb = ps.tile([C, T], F32, tag="big")
                for h in range(H2):
                    nc.tensor.matmul(rb[:, HLF[h]], ones1[:], r32(rft[0:1, HLF[h]]),
                                     start=True, stop=True, skip_group_check=True)
                xr = wka.tile([C, T], F32, tag="xr")
                nc.vector.tensor_mul(r32(xr[:]), xj[:], rb[:])

                xzh = ps.tile([D_INNER, T], F32, tag="big")
                xzz = ps.tile([D_INNER, T], F32, tag="big")
                for h in range(H2):
                    nc.tensor.matmul(xzh[:, HLF[h]], r32(W2T[:, 0:D_INNER]),
                                     r32(xr[:, HLF[h]]),
                                     start=True, stop=False, skip_group_check=True)
                    nc.tensor.matmul(xzh[:, HLF[h]], r32(w1n[:, 0:D_INNER]),
                                     r32(qft[0:1, HLF[h]]),
                                     start=False, stop=True, skip_group_check=True)
                    nc.tensor.matmul(xzz[:, HLF[h]], r32(W2T[:, D_INNER:2 * D_INNER]),
                                     r32(xr[:, HLF[h]]),
                                     start=True, stop=False, skip_group_check=True)
                    nc.tensor.matmul(xzz[:, HLF[h]], r32(w1n[:, D_INNER:2 * D_INNER]),
                                     r32(qft[0:1, HLF[h]]),
                                     start=False, stop=True, skip_group_check=True)

                # z gate: sz = silu(xzz + bz)
                sz = wka.tile([D_INNER, T], BF16, tag="sz")
                nc.scalar.activation(sz[:], xzz[:], AF.Silu, bias=bzp[:])

                # conv input (bf16) with 3-tap left halo
                cv = cvp.tile([D_INNER, T + 3], BF16, tag="cv")
                if j == 0:
                    nc.vector.memset(cv[:, 0:3], 0.0)
                else:
                    nc.vector.tensor_copy(cv[:, 0:3], prev_cv[:, T:T + 3])
                nc.scalar.activation(cv[:, 3:T + 3], xzh[:], AF.Copy)
                prev_cv = cv

                cps = ps.tile([D_INNER, T], F32, tag="big")
                for h in range(H2):
                    for k in range(4):
                        nc.tensor.matmul(cps[:, HLF[h]],
                                         diag[:, k * D_INNER:(k + 1) * D_INNER],
                                         cv[:, k + h * 512:k + h * 512 + 512],
                                         start=(k == 0), stop=(k == 3),
                                         skip_group_check=True)
                xh = wka.tile([D_INNER, T], BF16, tag="xh")
                nc.scalar.activation(xh[:], cps[:], AF.Silu, bias=cbp[:])

                # x_proj (packed dt|B|C) then copy out of PSUM as bf16
                dbl = ps.tile([DT_RANK + 2 * NS, T], F32, tag="big")
                for h in range(H2):
                    nc.tensor.matmul(dbl[:, HLF[h]], Wxp[:], xh[:, HLF[h]],
                                     start=True, stop=True, skip_group_check=True)
                dblc = wka.tile([DT_RANK + 2 * NS, T], BF16, tag="dblc")
                nc.scalar.activation(dblc[:], dbl[:], AF.Copy)
                nc.sync.dma_start(bcd_dram[:, sl], dblc[DT_RANK:DT_RANK + 2 * NS, :])

                dtp = ps.tile([D_INNER, T], F32, tag="big")
                for h in range(H2):
                    nc.tensor.matmul(dtp[:, HLF[h]], WdtT[:], dblc[0:DT_RANK, HLF[h]],
                                     start=True, stop=True, skip_group_check=True)

                # dt = softplus(dtp + dtb) = ln(1 + exp(dtp + dtb))
                esp = wka.tile([D_INNER, T], F32, tag="esp")
                nc.scalar.activation(esp[:], dtp[:], AF.Exp, bias=dtb[:])
                nc.vector.tensor_scalar_add(esp[:], esp[:], 1.0)
                dt = wka.tile([D_INNER, T], BF16, tag="dt")
                nc.scalar.activation(dt[:], esp[:], AF.Ln)

                w = wka.tile([D_INNER, T], BF16, tag="w")
                nc.gpsimd.tensor_mul(w[:], dt[:], xh[:])

                yacc = pacc.tile([D_INNER, T], F32, tag="yacc")
                prevP = None
                for n in range(NS):
                    dA = wka.tile([D_INNER, T], F32, tag="dA")
                    nc.scalar.activation(dA[:], dt[:], AF.Exp, bias=0.0,
                                         scale=Aneg[:, n:n + 1])
                    Bb = wkb.tile([D_INNER, T], BF16, tag="Bb")
                    nc.sync.dma_start(Bb[:], bcd_dram[n:n + 1, sl].broadcast_to([D_INNER, T]))
                    Cb = wkb.tile([D_INNER, T], BF16, tag="Cb")
                    nc.sync.dma_start(Cb[:], bcd_dram[NS + n:NS + n + 1, sl].broadcast_to([D_INNER, T]))
                    dBu = wkb.tile([D_INNER, T], BF16, tag="dBu")
                    nc.gpsimd.tensor_mul(dBu[:], w[:], Bb[:])
                    init = 0.0 if j == 0 else carry[:, n:n + 1]
                    nc.vector.tensor_tensor_scan(hbuf[:, n, :], dA[:], dBu[:], init,
                                                 OP.mult, OP.add)
                    if prevP is not None:
                        pn, Pt = prevP
                        for h in range(H2):
                            nc.tensor.matmul(yacc[:, HLF[h]], I128[:], Pt[:, HLF[h]],
                                             start=(pn == 0), stop=False,
                                             skip_group_check=True)
                    P = wkb.tile([D_INNER, T], BF16, tag="P")
                    nc.vector.tensor_mul(P[:], hbuf[:, n, :], Cb[:])
                    prevP = (n, P)
                # batched h carry for next chunk, then flush last P
                carry = wka.tile([D_INNER, NS], BF16, tag="carry")
                nc.vector.tensor_copy(carry[:], hbuf[:, :, T - 1:T])
                pn, Pt = prevP
                for h in range(H2):
                    nc.tensor.matmul(yacc[:, HLF[h]], I128[:], Pt[:, HLF[h]],
                                     start=False, stop=True, skip_group_check=True)

                y = wka.tile([D_INNER, T], BF16, tag="y")
                nc.vector.scalar_tensor_tensor(y[:], xh[:], Dsk[:], yacc[:],
                                               OP.mult, OP.add)
                gated = wka.tile([D_INNER, T], BF16, tag="gated")
                nc.gpsimd.tensor_mul(gated[:], y[:], sz[:])
                opj = ps.tile([C, T], F32, tag="big")
                for h in range(H2):
                    nc.tensor.matmul(opj[:, HLF[h]], WoutT[:], gated[:, HLF[h]],
                                     start=True, stop=True, skip_group_check=True)
                nc.vector.tensor_add(r32(fbuf[:, sl]), xj[:], opj[:])

                f2 = wka.tile([C, T], F32, tag="xr")
                nc.scalar.activation(r32(f2[:]), fbuf[:, sl], AF.Square)
                ln_stats(fbuf[:, sl], f2, st2_dram, j)

            stats_finish(st2_dram, r2_dram, q2_dram)

            # ================= final phase: LN2 + MLP (gelu) =========
            for j in range(NCH):
                sl = slice(j * T, (j + 1) * T)
                rft2 = wka.tile([1, T], F32, tag="rft")
                nc.sync.dma_start(r32(rft2[:]), r32(r2_dram[0:1, sl]))
                qft2 = wka.tile([1, T], F32, tag="qft")
                nc.sync.dma_start(r32(qft2[:]), r32(q2_dram[0:1, sl]))
                rb2 = ps.tile([C, T], F32, tag="big")
                for h in range(H2):
                    nc.tensor.matmul(rb2[:, HLF[h]], ones1[:], r32(rft2[0:1, HLF[h]]),
                                     start=True, stop=True, skip_group_check=True)
                fr = wka.tile([C, T], BF16, tag="fr")
                nc.vector.tensor_mul(fr[:], fbuf[:, sl], rb2[:])
                gtiles = []
                for g_i in range(2):
                    gp = ps.tile([2 * C, T], F32, tag="big")
                    for h in range(H2):
                        nc.tensor.matmul(gp[:, HLF[h]],
                                         Wfc1T[:, g_i * 2 * C:(g_i + 1) * 2 * C],
                                         fr[:, HLF[h]], start=True, stop=False,
                                         skip_group_check=True)
                        nc.tensor.matmul(gp[:, HLF[h]],
                                         r32(w1f[:, g_i * 2 * C:(g_i + 1) * 2 * C]),
                                         r32(qft2[0:1, HLF[h]]), start=False, stop=True,
                                         skip_group_check=True)
                    g = wka.tile([2 * C, T], BF16, tag=f"g{g_i}", bufs=1)
                    nc.scalar.activation(g[:], gp[:], AF.Gelu, bias=bfc1[:, g_i:g_i + 1])
                    gtiles.append(g)
                f2p = ps.tile([C, T], F32, tag="big")
                for h in range(H2):
                    nc.tensor.matmul(f2p[:, HLF[h]], Wfc2a[:], gtiles[0][:, HLF[h]],
                                     start=True, stop=False, skip_group_check=True)
                    nc.tensor.matmul(f2p[:, HLF[h]], Wfc2b[:], gtiles[1][:, HLF[h]],
                                     start=False, stop=True, skip_group_check=True)
                outf = wka.tile([C, T], F32, tag="esp")
                nc.vector.tensor_add(outf[:], fbuf[:, sl], f2p[:])
                nc.sync.dma_start(out_d[:, sl], outf[:])

    nc.compile()
    return nc


def _get_program():
    if "nc" not in _PROG:
        _PROG["nc"] = _build_program()
    return _PROG["nc"]


# ----------------------------------------------------------------------------
# host weight preprocessing per direction
# ----------------------------------------------------------------------------

def _bf16_dtype():
    try:
        import ml_dtypes
        return ml_dtypes.bfloat16
    except ImportError:
        import jax.numpy as jnp
        return jnp.bfloat16


def _prep_weights(li, inputs):
    f32 = np.float32
    bf16 = _bf16_dtype()
    in_w = np.asarray(inputs["in_proj_w"][li], np.float64)
    nw = np.asarray(inputs["norm_w"][li], np.float64)
    nb = np.asarray(inputs["norm_b"][li], np.float64)
    W2 = in_w * nw[None, :]
    bz_full = in_w @ nb
    b_h, b_z = bz_full[:D_INNER], bz_full[D_INNER:]
    cw = np.asarray(inputs["conv_w"][li], np.float64)
    cb = np.asarray(inputs["conv_b"][li], np.float64)
    cbtot = cb + b_h * cw.sum(1)
    diag = np.zeros((D_INNER, 4, D_INNER), np.float64)
    kk = np.arange(D_INNER)
    for k in range(4):
        diag[kk, k, kk] = cw[:, k]
    xp = np.asarray(inputs["x_proj_w"][li], np.float64)
    fc1 = np.asarray(inputs["fc1_w"], np.float64)
    fw = np.asarray(inputs["fnorm_w"], np.float64)
    fb = np.asarray(inputs["fnorm_b"], np.float64)
    fc1p = fc1 * fw[None, :]
    bfc1 = fc1 @ fb
    fc2 = np.asarray(inputs["fc2_w"], np.float64)
    return {
        "W2T": W2.T.astype(f32),
        "w1n": (-W2.sum(1))[None, :].astype(f32),
        "diag": diag.reshape(D_INNER, 4 * D_INNER).astype(bf16),
        "cbp": cbtot[:, None].astype(f32),
        "bzp": b_z[:, None].astype(f32),
        "Wxp": xp.T.astype(bf16),
        "WdtT": np.asarray(inputs["dt_proj_w"][li], np.float64).T.astype(bf16),
        "dtb": np.asarray(inputs["dt_proj_b"][li], f32)[:, None],
        "Aneg": (-np.exp(np.asarray(inputs["A_log"][li], np.float64))).astype(f32),
        "Dsk": np.asarray(inputs["D_skip"][li], f32)[:, None],
        "WoutT": np.asarray(inputs["out_proj_w"][li], np.float64).T.astype(bf16),
        "Wfc1T": fc1p.T.astype(bf16),
        "w1f": (-fc1p.sum(1))[None, :].astype(f32),
        "bfc1": bfc1.reshape(2, 128).T.astype(f32),
        "Wfc2a": fc2.T[:2 * C].astype(bf16),
        "Wfc2b": fc2.T[2 * C:].astype(bf16),
        "mv": np.full((C, 1), 1.0 / C, f32),
        "ones1": np.ones((1, C), f32),
        "epsb": np.full((NCH, 1), EPS, f32),
        "I128": np.eye(D_INNER).astype(bf16),
    }


def _reference_np(**inputs):
    """Pure-numpy fallback replica of the reference (slow, exact)."""
    i = int(inputs["src_number"]) % 4
    s = int(inputs["step_size"])
    feats, ori_h, ori_w = _rcds_np(np.asarray(inputs["ref_feat"], np.float32),
                                   np.asarray(inputs["src_feat"], np.float32), s, i)
    Bb, K, Cc, Ll = feats.shape
    f = feats.astype(np.float64)
    outs = np.empty_like(f)
    for d in range(4):
        li = d
        x = f[:, d].transpose(0, 2, 1)  # (B,L,C)
        mu = x.mean(-1, keepdims=True)
        var = ((x - mu) ** 2).mean(-1, keepdims=True)
        h = (x - mu) / np.sqrt(var + EPS) * np.asarray(inputs["norm_w"][li]) \
            + np.asarray(inputs["norm_b"][li])
        xz = h @ np.asarray(inputs["in_proj_w"][li]).T
        xh, z = xz[..., :D_INNER], xz[..., D_INNER:]
        xpd = np.pad(xh.transpose(0, 2, 1), ((0, 0), (0, 0), (3, 0)))
        cw = np.asarray(inputs["conv_w"][li])
        xc = sum(cw[:, k:k + 1] * xpd[:, :, k:k + Ll] for k in range(4))
        xc = xc + np.asarray(inputs["conv_b"][li])[None, :, None]
        xh = (xc / (1 + np.exp(-xc))).transpose(0, 2, 1)
        dbl = xh @ np.asarray(inputs["x_proj_w"][li]).T
        dtv = dbl[..., :DT_RANK]
        Bm = dbl[..., DT_RANK:DT_RANK + D_STATE]
        Cm = dbl[..., DT_RANK + D_STATE:]
        dtp = dtv @ np.asarray(inputs["dt_proj_w"][li]).T + np.asarray(inputs["dt_proj_b"][li])
        dtv = np.logaddexp(0, dtp)
        A = -np.exp(np.asarray(inputs["A_log"][li], np.float64))
        dA = np.exp(dtv[..., None] * A)
        dBu = (dtv * xh)[..., None] * Bm[:, :, None, :]
        hst = np.zeros((Bb, D_INNER, D_STATE))
        ys = np.empty((Bb, Ll, D_INNER))
        for t in range(Ll):
            hst = dA[:, t] * hst + dBu[:, t]
            ys[:, t] = np.einsum("bdn,bn->bd", hst, Cm[:, t])
        ys = ys + xh * np.asarray(inputs["D_skip"][li])
        ys = ys * (z / (1 + np.exp(-z)))
        o = ys @ np.asarray(inputs["out_proj_w"][li]).T
        outs[:, d] = (x + o).transpose(0, 2, 1)
    x = outs.transpose(0, 1, 3, 2)  # (B,4,L,C)
    mu = x.mean(-1, keepdims=True)
    var = ((x - mu) ** 2).mean(-1, keepdims=True)
    h = (x - mu) / np.sqrt(var + EPS) * np.asarray(inputs["fnorm_w"]) \
        + np.asarray(inputs["fnorm_b"])
    from scipy.special import erf
    g = h @ np.asarray(inputs["fc1_w"]).T
    g = 0.5 * g * (1 + erf(g / np.sqrt(2)))
    x = x + g @ np.asarray(inputs["fc2_w"]).T
    d0, d1 = _merge_np(x.transpose(0, 1, 3, 2).astype(np.float32), ori_h, ori_w, s, i)
    return d0.astype(np.float32), d1.astype(np.float32)


def kernel(**inputs):
    s = int(inputs["step_size"])
    i = int(inputs["src_number"]) % 4
    ref_feat = np.asarray(inputs["ref_feat"], np.float32)
    src_feat = np.asarray(inputs["src_feat"], np.float32)

    if s != 2 or ref_feat.shape != (B, C, HIMG, WIMG):
        return _reference_np(**inputs)

    feats, ori_h, ori_w = _rcds_np(ref_feat, src_feat, s, i)  # (B,4,C,L)

    wmaps = [_prep_weights(d, inputs) for d in range(4)]
    in_maps = []
    for core in range(8):
        b, d = core // 4, core % 4
        m = dict(wmaps[d])
        m["x"] = np.ascontiguousarray(feats[b, d])
        in_maps.append(m)

    from concourse.bass_utils import run_bass_kernel_spmd
    nc = _get_program()
    res = run_bass_kernel_spmd(nc, in_maps, list(range(8)))

    ys = np.empty((B, 4, C, L), np.float32)
    for core in range(8):
        b, d = core // 4, core % 4
        ys[b, d] = res.results[core]["out"]

    d0, d1 = _merge_np(ys, ori_h, ori_w, s, i)
    return d0, d1
